# revision 6
# baseline (speedup 1.0000x reference)
"""AttentionBlock Trainium2 kernel.

Reference computation (per batch b):
    xf = x[b].reshape(N, C);  N = 64*64 = 4096, C = 256, d = C//8 = 32
    q = xf @ Wq + bq; k = xf @ Wk + bk; v = xf @ Wv + bv
    out = softmax(q @ k.T) @ v
    y = gamma * out + xf

Sharding: 8 cores = 4 batches x 2 halves of the query rows. Each core
computes k/v for its full batch and attention for its 2048 query rows.

Per-core kernel design:
  - x arrives transposed (channels on partitions) in fp16; all projection
    matmuls contract channels on the partition dim in fp16 (error ~2e-4).
  - q/k are projected with 4x-replicated weights so all four 32-partition
    groups hold a full copy of q/k (fp16 in SBUF).
  - Scores are computed TRANSPOSED (scoresT[m, n] = k[m].q[n]) via 4-way
    PE row tiling: four concurrent K=32 matmuls (tile_position=(32i, 0)),
    one key tile per 32-row strip, ~3x faster than one K=128 matmul.
  - exp() is split across engines: ACT does strips 0-1 (table exp, bf16
    out), DVE does strips 2-3 with the Schraudolph bit trick (s*A+B ->
    int16, bitcast as bf16 ~= exp(s), ~1% rms on attention weights).
  - attn@v runs in bf16 (same PE rate as fp32, FWL-fast weight loads);
    gamma is folded into Wv on the host.
  - The exp'd scores are already layed out as the stationary operand of
    the attn@v matmul; output lands in natural [n, c] layout.
  - v is augmented with a ones column, so the attn@v accumulation also
    produces the softmax denominator (column 256) for free.
  - v bias passes through softmax exactly (attn rows sum to 1), so bv is
    folded into the host-prepared residual: xres' = x + gamma*bv.
  - xres / y use a partition-major DRAM layout so DMA packets are large
    (the natural [n, c] layout scatters into 1KB packets at ~37 GB/s).
  - PSUM: scores group [128,4,512] = 4 banks (one bank per strip,
    single-buffered) + attn@v accumulator [128,4,512] = 4 banks.
"""

import numpy as np

CH = 256
DQK = 32
N = 4096  # H*W
NQ = 2048  # query rows per core
B = 4
N_CORES = 8
CH2 = CH + 2  # v augmented with [denominator-ones, pad] columns
NS = 4  # query slices of 512
NG = 8  # groups of 4 key tiles
MT = N // 128  # 32 key tiles

# Schraudolph fast-exp constants, bf16 flavor:
# bitcast_bf16(int16(A*x + B)) ~= exp(x)  (~1% rms on attention weights)
EXP_A = 128.0 / float(np.log(2.0))
EXP_B = 127.0 * 128.0 - 6.0

_COMPILED = {}


def _build():
    """Build + compile the single-program SPMD Bass kernel. Cached."""
    if "nc" in _COMPILED:
        return _COMPILED["nc"]

    import concourse.bass as bass
    import concourse.tile as tile
    from concourse import bacc, mybir

    f32 = mybir.dt.float32
    bf16 = mybir.dt.bfloat16
    f16 = mybir.dt.float16
    i16 = mybir.dt.int16
    AF = mybir.ActivationFunctionType
    OP = mybir.AluOpType

    nc = bacc.Bacc(
        "TRN2",
        target_bir_lowering=False,
        debug=False,
        enable_asserts=True,
        num_devices=N_CORES,
    )

    # ---- I/O ----
    xT16 = nc.dram_tensor("xT16", [CH, N], f16, kind="ExternalInput").ap()
    # partition-major residual: xres[p, 256*t + c] = x[128*t + p, c] + g*bv[c]
    xres = nc.dram_tensor("xres", [128, NQ * CH // 128], f32, kind="ExternalInput").ap()
    cb16a_d = nc.dram_tensor("cb16a", [128, 512], f16, kind="ExternalInput").ap()
    cb16b_d = nc.dram_tensor("cb16b", [128, 512], f16, kind="ExternalInput").ap()
    cb32_d = nc.dram_tensor("cb32", [128, 4], f32, kind="ExternalInput").ap()
    # partition-major output, same layout as xres
    y = nc.dram_tensor("y", [128, NQ * CH // 128], f32, kind="ExternalOutput").ap()

    with tile.TileContext(nc) as tc:
        with (
            tc.tile_pool(name="consts", bufs=1) as consts,
            tc.tile_pool(name="xtp", bufs=1) as xtp,
            tc.tile_pool(name="qk", bufs=1) as qkp,
            tc.tile_pool(name="vp", bufs=1) as vp,
            tc.tile_pool(name="xrp", bufs=1) as xrp,
            tc.tile_pool(name="expp", bufs=3) as expp,
            tc.tile_pool(name="scrp", bufs=2) as scrp,
            tc.tile_pool(name="yp", bufs=2) as yp,
            tc.tile_pool(name="smallp", bufs=8) as smallp,
        ):
            # ---- constants (two small DMAs first) + x loads ----
            cb32 = consts.tile([128, 4], f32)
            nc.sync.dma_start(cb32[:], cb32_d[:, :])
            cb16a = consts.tile([128, 512], f16)
            nc.sync.dma_start(cb16a[:], cb16a_d[:, :])
            cb16b = consts.tile([128, 512], f16)
            nc.sync.dma_start(cb16b[:], cb16b_d[:, :])
            wq16 = lambda kt: cb16a[:, 128 * kt : 128 * (kt + 1)]
            wk16 = lambda kt: cb16a[:, 256 + 128 * kt : 256 + 128 * (kt + 1)]
            wv16 = lambda kt: cb16b[:, 256 * kt : 256 * (kt + 1)]
            bq4s = cb32[:, 0:1]
            bk4s = cb32[:, 1:2]

            # whole-row xts transfers: 8KB DMA packets (column-chunked loads
            # are 2KB-packet bound at ~91 B/ns; full rows hit ~350 B/ns, so
            # everything lands by ~10.5us, earlier than any progressive plan)
            xts = xtp.tile([128, 2, N], f16)
            xTr = xT16.rearrange("(t p) n -> p t n", p=128)
            nc.scalar.dma_start(xts[:, 0, :], xTr[:, 0, :])
            nc.gpsimd.dma_start(xts[:, 1, :], xTr[:, 1, :])

            xr = xrp.tile([128, NQ // 128, CH], f32)
            nc.sync.dma_start(
                xr[:], xres.rearrange("p (t c) -> p t c", c=CH)[:, :, :]
            )

            qt4 = qkp.tile([128, NQ], f16)
            kt4 = qkp.tile([128, N], f16)
            vaug = vp.tile([128, MT, CH2], bf16)
            # denominator-ones column + pad (written once; v copies fill 0:256)
            nc.gpsimd.memset(vaug[:, :, 256:257], 1.0)
            nc.gpsimd.memset(vaug[:, :, 257:258], 0.0)

            # ---- warmup (HAM clock ramp) + gamma broadcast + projections ----
            with (
                tc.tile_pool(name="psqk", bufs=3, space="PSUM") as psqk,
                tc.tile_pool(name="psv", bufs=4, space="PSUM") as psv,
            ):
                warm_sink = consts.tile([128, 1], f32)
                for w in range(6):
                    wt = psqk.tile([128, 512], f32, tag="pqk", name=f"warm{w}")
                    nc.tensor.matmul(
                        wt[:],
                        lhsT=cb16a[:, 0:128],
                        rhs=cb16a[:, 0:512],
                        start=True,
                        stop=True,
                    )
                    if w == 5:
                        # keep the chain observable so it isn't dead-code
                        nc.vector.tensor_reduce(
                            warm_sink[:], wt[:], axis=mybir.AxisListType.X,
                            op=OP.max,
                        )
                # tiny exp so the ACT table set loads here (ACT is idle),
                # not right before the first real exp
                warm_exp = consts.tile([1, 2], f32)
                nc.scalar.activation(warm_exp[:], cb32[0:1, 0:2], AF.Exp)

                # ---- projections (fp16 in, f32 PSUM) ----
                # qT4[32a+d, n] = q[n, d] (own half), replicated over a
                for t in range(NS):
                    pq = psqk.tile([128, 512], f32, tag="pqk", name=f"pq{t}")
                    for kt in range(2):
                        nc.tensor.matmul(
                            pq[:],
                            lhsT=wq16(kt),
                            rhs=xts[:, kt, 512 * t : 512 * (t + 1)],
                            start=(kt == 0),
                            stop=(kt == 1),
                        )
                    nc.scalar.activation(
                        qt4[:, 512 * t : 512 * (t + 1)], pq[:],
                        AF.Identity, bias=bq4s,
                    )
                # kT4 over the full batch
                for t in range(N // 512):
                    pk = psqk.tile([128, 512], f32, tag="pqk", name=f"pk{t}")
                    for kt in range(2):
                        nc.tensor.matmul(
                            pk[:],
                            lhsT=wk16(kt),
                            rhs=xts[:, kt, 512 * t : 512 * (t + 1)],
                            start=(kt == 0),
                            stop=(kt == 1),
                        )
                    nc.scalar.activation(
                        kt4[:, 512 * t : 512 * (t + 1)], pk[:],
                        AF.Identity, bias=bk4s,
                    )
                # v natural layout [key, c]; bias folded into host residual
                for mt in range(MT):
                    pv = psv.tile([128, 512], f32, tag="pv", name=f"pv{mt}")
                    for kt in range(2):
                        nc.tensor.matmul(
                            pv[:, 0:CH],
                            lhsT=xts[:, kt, 128 * mt : 128 * (mt + 1)],
                            rhs=wv16(kt),
                            start=(kt == 0),
                            stop=(kt == 1),
                        )
                    nc.vector.tensor_copy(vaug[:, mt, 0:CH], pv[:, 0:CH])

            # ---- attention main loop ----
            # Per group of 4 key tiles: 4 concurrent row-tiled K=32 score
            # matmuls (one PSUM bank each, single-buffered), exp split
            # ACT/DVE, then 16 attn@v accumulation matmuls. PE emission
            # order is scores(g+1) before attnv(g) so the exp latency hides
            # under the previous group's attn@v stream.
            with (
                tc.tile_pool(name="pss", bufs=1, space="PSUM") as pss,
                tc.tile_pool(name="psa", bufs=1, space="PSUM") as psa,
            ):
                def scores_mm(ns, g, s):
                    for i in range(4):
                        mt = 4 * g + i
                        nc.tensor.matmul(
                            s[:, i, :],
                            lhsT=kt4[32 * i : 32 * (i + 1), 128 * mt : 128 * (mt + 1)],
                            rhs=qt4[32 * i : 32 * (i + 1), 512 * ns : 512 * (ns + 1)],
                            start=True,
                            stop=True,
                            tile_position=(32 * i, 0),
                        )

                def attnv(e, g, acc):
                    # j-outer on the last group: each j's accumulation chain
                    # stops as early as possible so normalize(j) overlaps the
                    # remaining attn@v matmuls
                    order = (
                        [(j, i) for j in range(4) for i in range(4)]
                        if g == NG - 1
                        else [(j, i) for i in range(4) for j in range(4)]
                    )
                    for j, i in order:
                        mt = 4 * g + i
                        nc.tensor.matmul(
                            acc[:, j, 0:CH2],
                            lhsT=e[:, i, 128 * j : 128 * (j + 1)],
                            rhs=vaug[:, mt, :],
                            start=(g == 0 and i == 0),
                            stop=(g == NG - 1 and i == 3),
                        )

                def normalize(ns, acc):
                    # per-j evacuation: j's chain stopped early (j-outer
                    # final group), so copy+normalize overlap the remaining
                    # attn@v work and release acc banks sooner
                    accs = yp.tile([128, 4, CH2], f32, tag="accs")
                    yt = yp.tile([128, 4, CH], f32, tag="yt")
                    for j in range(4):
                        nc.vector.tensor_copy(accs[:, j, :], acc[:, j, 0:CH2])
                        r = smallp.tile([128, 1], f32)
                        nc.vector.reciprocal(r[:], accs[:, j, CH : CH + 1])
                        nc.vector.scalar_tensor_tensor(
                            yt[:, j, :],
                            accs[:, j, 0:CH],
                            r[:, 0:1],
                            xr[:, 4 * ns + j, :],
                            op0=OP.mult,
                            op1=OP.add,
                        )
                    nc.gpsimd.dma_start(
                        y.rearrange("p (t c) -> p t c", c=CH)[
                            :, 4 * ns : 4 * (ns + 1), :
                        ],
                        yt[:],
                    )

                prev = None
                acc = None
                for ns in range(NS):
                    for g in range(NG):
                        s = pss.tile([128, 4, 512], f32, tag="s", name=f"s{ns}_{g}")
                        scores_mm(ns, g, s)
                        if prev is not None:
                            pe_, pns, pg_, pacc = prev
                            attnv(pe_, pg_, pacc)
                            if pg_ == NG - 1:
                                normalize(pns, pacc)
                        if g == 0:
                            acc = psa.tile([128, 4, 512], f32, tag="acc", name=f"acc{ns}")
                        e = expp.tile([128, 4, 512], bf16, tag="e", name=f"e{ns}_{g}")
                        # per-strip instructions so attn@v strip i only waits
                        # on its own exp, and the score banks release early.
                        # Strips 2-3: DVE computes A*s+B into f32 scratch
                        # (PSUM read; no cast -> single instr), then GpSimd
                        # (SBUF-only engine) convert-copies to the int16 bf16
                        # bit pattern.
                        nc.scalar.activation(e[:, 0, :], s[:, 0, :], AF.Exp)
                        nc.scalar.activation(e[:, 1, :], s[:, 1, :], AF.Exp)
                        scr = scrp.tile([128, 2, 512], f32, tag="scr")
                        nc.vector.tensor_scalar(
                            scr[:], s[:, 2:4, :],
                            EXP_A, EXP_B, op0=OP.mult, op1=OP.add,
                        )
                        nc.gpsimd.tensor_copy(e[:, 2:4, :].bitcast(i16), scr[:])
                        prev = (e, ns, g, acc)
                pe_, pns, pg_, pacc = prev
                attnv(pe_, pg_, pacc)
                normalize(pns, pacc)

    nc.compile()
    _COMPILED["nc"] = nc
    return nc


def _pack_consts(Wq, bq, Wk, bk, Wv, bv, gamma):
    """Pack constants into fp16 weight blob + fp32 small blob.

    cb16 (per partition p):
      [0:256)    Wq4 k-tiles: [wq4[p], wq4[p+128]]  (wq4 = tile(Wq, (1,4)))
      [256:512)  Wk4 k-tiles
      [512:1024) gamma*Wv k-tiles (256 each; gamma folded so the attn@v
                 accumulator is already scaled -- no gamma on device)
    cb32: [:,0]=bq4, [:,1]=bk4
    """
    g = np.float32(np.asarray(gamma).reshape(()))
    Wq4 = np.tile(np.asarray(Wq, np.float32), (1, 4)).astype(np.float16)
    Wk4 = np.tile(np.asarray(Wk, np.float32), (1, 4)).astype(np.float16)
    Wv16 = (g * np.asarray(Wv, np.float32)).astype(np.float16)

    cb16a = np.zeros((128, 512), np.float16)
    cb16b = np.zeros((128, 512), np.float16)
    for kt in range(2):
        cb16a[:, 128 * kt : 128 * (kt + 1)] = Wq4[128 * kt : 128 * (kt + 1), :]
        cb16a[:, 256 + 128 * kt : 256 + 128 * (kt + 1)] = Wk4[128 * kt : 128 * (kt + 1)]
        cb16b[:, 256 * kt : 256 * (kt + 1)] = Wv16[128 * kt : 128 * (kt + 1)]
    cb32 = np.zeros((128, 4), np.float32)
    cb32[:, 0] = np.tile(np.asarray(bq, np.float32), 4)
    cb32[:, 1] = np.tile(np.asarray(bk, np.float32), 4)
    return cb16a, cb16b, cb32


def _shard_inputs(x, Wq, bq, Wk, bk, Wv, bv, gamma):
    """Host-side prep: one input map per core."""
    xf = np.ascontiguousarray(x, dtype=np.float32).reshape(B, N, CH)
    x16 = xf.astype(np.float16)
    cb16a, cb16b, cb32 = _pack_consts(Wq, bq, Wk, bk, Wv, bv, gamma)
    g = np.float32(np.asarray(gamma).reshape(()))
    bv32 = np.asarray(bv, np.float32)

    in_maps = []
    for c in range(N_CORES):
        b, h = divmod(c, 2)
        own = slice(h * NQ, (h + 1) * NQ)
        other = slice((1 - h) * NQ, (2 - h) * NQ)
        xT = np.concatenate([x16[b, own].T, x16[b, other].T], axis=1)
        # partition-major residual with folded v bias
        xr = xf[b, own] + g * bv32[None, :]
        xr_p = xr.reshape(NQ // 128, 128, CH).transpose(1, 0, 2).reshape(128, -1)
        in_maps.append(
            {
                "xT16": np.ascontiguousarray(xT),
                "xres": np.ascontiguousarray(xr_p),
                "cb16a": cb16a,
                "cb16b": cb16b,
                "cb32": cb32,
            }
        )
    return in_maps


def _unshard(results, shape):
    out = np.empty((B, N, CH), np.float32)
    for c in range(N_CORES):
        b, h = divmod(c, 2)
        yp = (
            results[c]["y"]
            .reshape(128, NQ // 128, CH)
            .transpose(1, 0, 2)
            .reshape(NQ, CH)
        )
        out[b, h * NQ : (h + 1) * NQ, :] = yp
    return out.reshape(shape)


def kernel(x, Wq, bq, Wk, bk, Wv, bv, gamma):
    from concourse.bass_utils import run_bass_kernel_spmd

    nc = _build()
    in_maps = _shard_inputs(x, Wq, bq, Wk, bk, Wv, bv, gamma)
    res = run_bass_kernel_spmd(nc, in_maps, core_ids=list(range(N_CORES)))
    return _unshard(res.results, x.shape)


# revision 8
# speedup vs baseline: 1.3812x; 1.3812x over previous
"""AttentionBlock Trainium2 kernel.

Reference computation (per batch b):
    xf = x[b].reshape(N, C);  N = 64*64 = 4096, C = 256, d = C//8 = 32
    q = xf @ Wq + bq; k = xf @ Wk + bk; v = xf @ Wv + bv
    out = softmax(q @ k.T) @ v
    y = gamma * out + xf

Sharding: 8 cores = 4 batches x 2 halves of the query rows. Each core
computes k/v for its full batch and attention for its 2048 query rows.

Per-core kernel design:
  - x arrives transposed (channels on partitions) in fp16; all projection
    matmuls contract channels on the partition dim in fp16 (error ~2e-4).
  - q/k are projected with 4x-replicated weights so all four 32-partition
    groups hold a full copy of q/k (fp16 in SBUF).
  - Scores are computed TRANSPOSED (scoresT[m, n] = k[m].q[n]) via 4-way
    PE row tiling: four concurrent K=32 matmuls (tile_position=(32i, 0)),
    one key tile per 32-row strip, ~3x faster than one K=128 matmul.
  - exp() is split across engines: ACT does strips 0-1 (table exp, bf16
    out), DVE does strips 2-3 with the Schraudolph bit trick (s*A+B ->
    int16, bitcast as bf16 ~= exp(s), ~1% rms on attention weights).
  - attn@v runs in bf16 (same PE rate as fp32, FWL-fast weight loads);
    gamma is folded into Wv on the host.
  - The exp'd scores are already layed out as the stationary operand of
    the attn@v matmul; output lands in natural [n, c] layout.
  - v is augmented with a ones column, so the attn@v accumulation also
    produces the softmax denominator (column 256) for free.
  - v bias passes through softmax exactly (attn rows sum to 1), so bv is
    folded into the host-prepared residual: xres' = x + gamma*bv.
  - xres / y use a partition-major DRAM layout so DMA packets are large
    (the natural [n, c] layout scatters into 1KB packets at ~37 GB/s).
  - PSUM: scores group [128,4,512] = 4 banks (one bank per strip,
    single-buffered) + attn@v accumulator [128,4,512] = 4 banks.
"""

import numpy as np

CH = 256
DQK = 32
N = 4096  # H*W
NQ = 2048  # query rows per core
B = 4
N_CORES = 8
CH2 = CH + 2  # v augmented with [denominator-ones, pad] columns
NS = 4  # query slices of 512
NG = 8  # groups of 4 key tiles
MT = N // 128  # 32 key tiles

# Schraudolph fast-exp constants, bf16 flavor:
# bitcast_bf16(int16(A*x + B)) ~= exp(x)  (~1% rms on attention weights)
EXP_A = 128.0 / float(np.log(2.0))
EXP_B = 127.0 * 128.0 - 6.0

_COMPILED = {}


def _build():
    """Build + compile the single-program SPMD Bass kernel. Cached."""
    if "nc" in _COMPILED:
        return _COMPILED["nc"]

    import concourse.bass as bass
    import concourse.tile as tile
    from concourse import bacc, mybir

    f32 = mybir.dt.float32
    bf16 = mybir.dt.bfloat16
    f16 = mybir.dt.float16
    i16 = mybir.dt.int16
    AF = mybir.ActivationFunctionType
    OP = mybir.AluOpType

    nc = bacc.Bacc(
        "TRN2",
        target_bir_lowering=False,
        debug=False,
        enable_asserts=True,
        num_devices=N_CORES,
    )

    # ---- I/O ----
    xT16 = nc.dram_tensor("xT16", [CH, N], f16, kind="ExternalInput").ap()
    # partition-major residual: xres[p, 256*t + c] = x[128*t + p, c] + g*bv[c]
    xres = nc.dram_tensor("xres", [128, NQ * CH // 128], f32, kind="ExternalInput").ap()
    cb16a_d = nc.dram_tensor("cb16a", [128, 512], f16, kind="ExternalInput").ap()
    cb16b_d = nc.dram_tensor("cb16b", [128, 512], f16, kind="ExternalInput").ap()
    cb32_d = nc.dram_tensor("cb32", [128, 4], f32, kind="ExternalInput").ap()
    # partition-major output, same layout as xres
    y = nc.dram_tensor("y", [128, NQ * CH // 128], f32, kind="ExternalOutput").ap()

    with tile.TileContext(nc) as tc:
        with (
            tc.tile_pool(name="consts", bufs=1) as consts,
            tc.tile_pool(name="xtp", bufs=1) as xtp,
            tc.tile_pool(name="qk", bufs=1) as qkp,
            tc.tile_pool(name="vp", bufs=1) as vp,
            tc.tile_pool(name="xrp", bufs=1) as xrp,
            tc.tile_pool(name="expp", bufs=3) as expp,
            tc.tile_pool(name="yp", bufs=2) as yp,
            tc.tile_pool(name="smallp", bufs=8) as smallp,
        ):
            # ---- constants (two small DMAs first) + x loads ----
            cb32 = consts.tile([128, 4], f32)
            nc.sync.dma_start(cb32[:], cb32_d[:, :])
            cb16a = consts.tile([128, 512], f16)
            nc.sync.dma_start(cb16a[:], cb16a_d[:, :])
            cb16b = consts.tile([128, 512], f16)
            nc.sync.dma_start(cb16b[:], cb16b_d[:, :])
            wq16 = lambda kt: cb16a[:, 128 * kt : 128 * (kt + 1)]
            wk16 = lambda kt: cb16a[:, 256 + 128 * kt : 256 + 128 * (kt + 1)]
            wv16 = lambda kt: cb16b[:, 256 * kt : 256 * (kt + 1)]
            bq4s = cb32[:, 0:1]
            bk4s = cb32[:, 1:2]

            # whole-row xts transfers: 8KB DMA packets (column-chunked loads
            # are 2KB-packet bound at ~91 B/ns; full rows hit ~350 B/ns, so
            # everything lands by ~10.5us, earlier than any progressive plan)
            xts = xtp.tile([128, 2, N], f16)
            xTr = xT16.rearrange("(t p) n -> p t n", p=128)
            nc.scalar.dma_start(xts[:, 0, :], xTr[:, 0, :])
            nc.gpsimd.dma_start(xts[:, 1, :], xTr[:, 1, :])

            xr = xrp.tile([128, NQ // 128, CH], f32)
            nc.scalar.dma_start(
                xr[:], xres.rearrange("p (t c) -> p t c", c=CH)[:, :, :]
            )

            qt4 = qkp.tile([128, NQ], f16)
            kt4 = qkp.tile([128, N], f16)
            vaug = vp.tile([128, MT, CH2], bf16)
            # denominator-ones column + pad (written once; v copies fill 0:256)
            nc.gpsimd.memset(vaug[:, :, 256:257], 1.0)
            nc.gpsimd.memset(vaug[:, :, 257:258], 0.0)

            # ---- warmup (HAM clock ramp) + gamma broadcast + projections ----
            with (
                tc.tile_pool(name="psqk", bufs=3, space="PSUM") as psqk,
                tc.tile_pool(name="psv", bufs=4, space="PSUM") as psv,
            ):
                warm_sink = consts.tile([128, 1], f32)
                for w in range(2):
                    wt = psqk.tile([128, 512], f32, tag="pqk", name=f"warm{w}")
                    nc.tensor.matmul(
                        wt[:],
                        lhsT=cb16a[:, 0:128],
                        rhs=cb16a[:, 0:512],
                        start=True,
                        stop=True,
                    )
                    if w == 1:
                        # keep the chain observable so it isn't dead-code
                        nc.vector.tensor_reduce(
                            warm_sink[:], wt[:], axis=mybir.AxisListType.X,
                            op=OP.max,
                        )
                # tiny exp so the ACT table set loads here (ACT is idle),
                # not right before the first real exp
                warm_exp = consts.tile([1, 2], f32)
                nc.scalar.activation(warm_exp[:], cb32[0:1, 0:2], AF.Exp)

                # ---- projections (fp16 in, f32 PSUM) ----
                # qT4[32a+d, n] = q[n, d] (own half), replicated over a
                for t in range(NS):
                    pq = psqk.tile([128, 512], f32, tag="pqk", name=f"pq{t}")
                    for kt in range(2):
                        nc.tensor.matmul(
                            pq[:],
                            lhsT=wq16(kt),
                            rhs=xts[:, kt, 512 * t : 512 * (t + 1)],
                            start=(kt == 0),
                            stop=(kt == 1),
                        )
                    nc.scalar.activation(
                        qt4[:, 512 * t : 512 * (t + 1)], pq[:],
                        AF.Identity, bias=bq4s,
                    )
                # kT4 over the full batch
                for t in range(N // 512):
                    pk = psqk.tile([128, 512], f32, tag="pqk", name=f"pk{t}")
                    for kt in range(2):
                        nc.tensor.matmul(
                            pk[:],
                            lhsT=wk16(kt),
                            rhs=xts[:, kt, 512 * t : 512 * (t + 1)],
                            start=(kt == 0),
                            stop=(kt == 1),
                        )
                    nc.scalar.activation(
                        kt4[:, 512 * t : 512 * (t + 1)], pk[:],
                        AF.Identity, bias=bk4s,
                    )
                # v natural layout [key, c]; bias folded into host residual
                for mt in range(MT):
                    pv = psv.tile([128, 512], f32, tag="pv", name=f"pv{mt}")
                    for kt in range(2):
                        nc.tensor.matmul(
                            pv[:, 0:CH],
                            lhsT=xts[:, kt, 128 * mt : 128 * (mt + 1)],
                            rhs=wv16(kt),
                            start=(kt == 0),
                            stop=(kt == 1),
                        )
                    nc.vector.tensor_copy(vaug[:, mt, 0:CH], pv[:, 0:CH])

            # ---- attention main loop ----
            # Per group of 4 key tiles: 4 concurrent row-tiled K=32 score
            # matmuls (one PSUM bank each, single-buffered), exp split
            # ACT/DVE, then 16 attn@v accumulation matmuls. PE emission
            # order is scores(g+1) before attnv(g) so the exp latency hides
            # under the previous group's attn@v stream.
            with (
                tc.tile_pool(name="pss", bufs=1, space="PSUM") as pss,
                tc.tile_pool(name="psa", bufs=1, space="PSUM") as psa,
            ):
                def scores_mm(ns, g, s):
                    for i in range(4):
                        mt = 4 * g + i
                        nc.tensor.matmul(
                            s[:, i, :],
                            lhsT=kt4[32 * i : 32 * (i + 1), 128 * mt : 128 * (mt + 1)],
                            rhs=qt4[32 * i : 32 * (i + 1), 512 * ns : 512 * (ns + 1)],
                            start=True,
                            stop=True,
                            tile_position=(32 * i, 0),
                        )

                def attnv(e, g, acc):
                    # j-outer on the last group: each j's accumulation chain
                    # stops as early as possible so normalize(j) overlaps the
                    # remaining attn@v matmuls
                    order = (
                        [(j, i) for j in range(4) for i in range(4)]
                        if g == NG - 1
                        else [(j, i) for i in range(4) for j in range(4)]
                    )
                    for j, i in order:
                        mt = 4 * g + i
                        nc.tensor.matmul(
                            acc[:, j, 0:CH2],
                            lhsT=e[:, i, 128 * j : 128 * (j + 1)],
                            rhs=vaug[:, mt, :],
                            start=(g == 0 and i == 0),
                            stop=(g == NG - 1 and i == 3),
                        )

                def normalize(ns, acc):
                    # per-j evacuation: j's chain stopped early (j-outer
                    # final group), so copy+normalize overlap the remaining
                    # attn@v work and release acc banks sooner
                    accs = yp.tile([128, 4, CH2], f32, tag="accs")
                    yt = yp.tile([128, 4, CH], f32, tag="yt")
                    for j in range(4):
                        # acc evacuation on ACT (it has slack; DVE is the
                        # tighter engine in the mainloop)
                        nc.scalar.activation(
                            accs[:, j, :], acc[:, j, 0:CH2], AF.Identity
                        )
                        r = smallp.tile([128, 1], f32)
                        nc.vector.reciprocal(r[:], accs[:, j, CH : CH + 1])
                        nc.vector.scalar_tensor_tensor(
                            yt[:, j, :],
                            accs[:, j, 0:CH],
                            r[:, 0:1],
                            xr[:, 4 * ns + j, :],
                            op0=OP.mult,
                            op1=OP.add,
                        )
                    nc.gpsimd.dma_start(
                        y.rearrange("p (t c) -> p t c", c=CH)[
                            :, 4 * ns : 4 * (ns + 1), :
                        ],
                        yt[:],
                    )

                prev = None
                acc = None
                for ns in range(NS):
                    for g in range(NG):
                        s = pss.tile([128, 4, 512], f32, tag="s", name=f"s{ns}_{g}")
                        scores_mm(ns, g, s)
                        if prev is not None:
                            pe_, pns, pg_, pacc = prev
                            attnv(pe_, pg_, pacc)
                            if pg_ == NG - 1:
                                normalize(pns, pacc)
                        if g == 0:
                            acc = psa.tile([128, 4, 512], f32, tag="acc", name=f"acc{ns}")
                        e = expp.tile([128, 4, 512], bf16, tag="e", name=f"e{ns}_{g}")
                        # per-strip instructions so attn@v strip i only waits
                        # on its own exp, and the score banks release early
                        nc.scalar.activation(e[:, 0, :], s[:, 0, :], AF.Exp)
                        nc.scalar.activation(e[:, 1, :], s[:, 1, :], AF.Exp)
                        nc.vector.tensor_scalar(
                            e[:, 2:4, :].bitcast(i16), s[:, 2:4, :],
                            EXP_A, EXP_B, op0=OP.mult, op1=OP.add,
                        )
                        prev = (e, ns, g, acc)
                pe_, pns, pg_, pacc = prev
                attnv(pe_, pg_, pacc)
                normalize(pns, pacc)

    nc.compile()
    _COMPILED["nc"] = nc
    return nc


def _pack_consts(Wq, bq, Wk, bk, Wv, bv, gamma):
    """Pack constants into fp16 weight blob + fp32 small blob.

    cb16 (per partition p):
      [0:256)    Wq4 k-tiles: [wq4[p], wq4[p+128]]  (wq4 = tile(Wq, (1,4)))
      [256:512)  Wk4 k-tiles
      [512:1024) gamma*Wv k-tiles (256 each; gamma folded so the attn@v
                 accumulator is already scaled -- no gamma on device)
    cb32: [:,0]=bq4, [:,1]=bk4
    """
    g = np.float32(np.asarray(gamma).reshape(()))
    Wq4 = np.tile(np.asarray(Wq, np.float32), (1, 4)).astype(np.float16)
    Wk4 = np.tile(np.asarray(Wk, np.float32), (1, 4)).astype(np.float16)
    Wv16 = (g * np.asarray(Wv, np.float32)).astype(np.float16)

    cb16a = np.zeros((128, 512), np.float16)
    cb16b = np.zeros((128, 512), np.float16)
    for kt in range(2):
        cb16a[:, 128 * kt : 128 * (kt + 1)] = Wq4[128 * kt : 128 * (kt + 1), :]
        cb16a[:, 256 + 128 * kt : 256 + 128 * (kt + 1)] = Wk4[128 * kt : 128 * (kt + 1)]
        cb16b[:, 256 * kt : 256 * (kt + 1)] = Wv16[128 * kt : 128 * (kt + 1)]
    cb32 = np.zeros((128, 4), np.float32)
    cb32[:, 0] = np.tile(np.asarray(bq, np.float32), 4)
    cb32[:, 1] = np.tile(np.asarray(bk, np.float32), 4)
    return cb16a, cb16b, cb32


def _shard_inputs(x, Wq, bq, Wk, bk, Wv, bv, gamma):
    """Host-side prep: one input map per core."""
    xf = np.ascontiguousarray(x, dtype=np.float32).reshape(B, N, CH)
    x16 = xf.astype(np.float16)
    cb16a, cb16b, cb32 = _pack_consts(Wq, bq, Wk, bk, Wv, bv, gamma)
    g = np.float32(np.asarray(gamma).reshape(()))
    bv32 = np.asarray(bv, np.float32)

    in_maps = []
    for c in range(N_CORES):
        b, h = divmod(c, 2)
        own = slice(h * NQ, (h + 1) * NQ)
        other = slice((1 - h) * NQ, (2 - h) * NQ)
        xT = np.concatenate([x16[b, own].T, x16[b, other].T], axis=1)
        # partition-major residual with folded v bias
        xr = xf[b, own] + g * bv32[None, :]
        xr_p = xr.reshape(NQ // 128, 128, CH).transpose(1, 0, 2).reshape(128, -1)
        in_maps.append(
            {
                "xT16": np.ascontiguousarray(xT),
                "xres": np.ascontiguousarray(xr_p),
                "cb16a": cb16a,
                "cb16b": cb16b,
                "cb32": cb32,
            }
        )
    return in_maps


def _unshard(results, shape):
    out = np.empty((B, N, CH), np.float32)
    for c in range(N_CORES):
        b, h = divmod(c, 2)
        yp = (
            results[c]["y"]
            .reshape(128, NQ // 128, CH)
            .transpose(1, 0, 2)
            .reshape(NQ, CH)
        )
        out[b, h * NQ : (h + 1) * NQ, :] = yp
    return out.reshape(shape)


def kernel(x, Wq, bq, Wk, bk, Wv, bv, gamma):
    from concourse.bass_utils import run_bass_kernel_spmd

    nc = _build()
    in_maps = _shard_inputs(x, Wq, bq, Wk, bk, Wv, bv, gamma)
    res = run_bass_kernel_spmd(nc, in_maps, core_ids=list(range(N_CORES)))
    return _unshard(res.results, x.shape)


# revision 9
# speedup vs baseline: 1.5463x; 1.1195x over previous
"""AttentionBlock Trainium2 kernel.

Reference computation (per batch b):
    xf = x[b].reshape(N, C);  N = 64*64 = 4096, C = 256, d = C//8 = 32
    q = xf @ Wq + bq; k = xf @ Wk + bk; v = xf @ Wv + bv
    out = softmax(q @ k.T) @ v
    y = gamma * out + xf

Sharding: 8 cores = 4 batches x 2 halves of the query rows. Each core
computes k/v for its full batch and attention for its 2048 query rows.

Per-core kernel design:
  - x arrives transposed (channels on partitions) in fp16; all projection
    matmuls contract channels on the partition dim in fp16 (error ~2e-4).
  - q/k are projected with 4x-replicated weights so all four 32-partition
    groups hold a full copy of q/k (fp16 in SBUF).
  - Scores are computed TRANSPOSED (scoresT[m, n] = k[m].q[n]) via 4-way
    PE row tiling: four concurrent K=32 matmuls (tile_position=(32i, 0)),
    one key tile per 32-row strip, ~3x faster than one K=128 matmul.
  - exp() is split across engines: ACT does strips 0-1 (table exp, bf16
    out), DVE does strips 2-3 with the Schraudolph bit trick (s*A+B ->
    int16, bitcast as bf16 ~= exp(s), ~1% rms on attention weights).
  - attn@v runs in bf16 (same PE rate as fp32, FWL-fast weight loads);
    gamma is folded into Wv on the host.
  - The exp'd scores are already layed out as the stationary operand of
    the attn@v matmul; output lands in natural [n, c] layout.
  - v is augmented with a ones column, so the attn@v accumulation also
    produces the softmax denominator (column 256) for free.
  - v bias passes through softmax exactly (attn rows sum to 1), so bv is
    folded into the host-prepared residual: xres' = x + gamma*bv.
  - xres / y use a partition-major DRAM layout so DMA packets are large
    (the natural [n, c] layout scatters into 1KB packets at ~37 GB/s).
  - PSUM: scores group [128,4,512] = 4 banks (one bank per strip,
    single-buffered) + attn@v accumulator [128,4,512] = 4 banks.
"""

import numpy as np

CH = 256
DQK = 32
N = 4096  # H*W
NQ = 2048  # query rows per core
B = 4
N_CORES = 8
CH2 = CH + 2  # v augmented with [denominator-ones, pad] columns
NS = 4  # query slices of 512
NG = 16  # groups of 2 key tiles
MT = N // 128  # 32 key tiles

# Schraudolph fast-exp constants, bf16 flavor:
# bitcast_bf16(int16(A*x + B)) ~= exp(x)  (~1% rms on attention weights)
EXP_A = 128.0 / float(np.log(2.0))
EXP_B = 127.0 * 128.0 - 6.0

_COMPILED = {}


def _build():
    """Build + compile the single-program SPMD Bass kernel. Cached."""
    if "nc" in _COMPILED:
        return _COMPILED["nc"]

    import concourse.bass as bass
    import concourse.tile as tile
    from concourse import bacc, mybir

    f32 = mybir.dt.float32
    bf16 = mybir.dt.bfloat16
    f16 = mybir.dt.float16
    i16 = mybir.dt.int16
    AF = mybir.ActivationFunctionType
    OP = mybir.AluOpType

    nc = bacc.Bacc(
        "TRN2",
        target_bir_lowering=False,
        debug=False,
        enable_asserts=True,
        num_devices=N_CORES,
    )

    # ---- I/O ----
    # xta row r (r<128):   [xT[r, 0:N], Wq4/Wk4 row r, biases, pad]
    # xta row 128+r:       [xT[128+r, 0:N], Wv row r, pad]
    # One 9KB-row tensor: DMA packet overhead makes separate small blobs
    # (1-2KB rows) take 10us+; whole 9KB rows stream at ~350 B/ns.
    XAUG = N + 512 + 8
    xta = nc.dram_tensor("xta", [CH, XAUG], f16, kind="ExternalInput").ap()
    # partition-major residual: xres[p, 256*t + c] = x[128*t + p, c] + g*bv[c]
    xres = nc.dram_tensor("xres", [128, NQ * CH // 128], f32, kind="ExternalInput").ap()
    # partition-major output, same layout as xres
    y = nc.dram_tensor("y", [128, NQ * CH // 128], f32, kind="ExternalOutput").ap()

    with tile.TileContext(nc) as tc:
        with (
            tc.tile_pool(name="consts", bufs=1) as consts,
            tc.tile_pool(name="xtp", bufs=1) as xtp,
            tc.tile_pool(name="qk", bufs=1) as qkp,
            tc.tile_pool(name="vp", bufs=1) as vp,
            tc.tile_pool(name="xrp", bufs=1) as xrp,
            tc.tile_pool(name="expp", bufs=3) as expp,
            tc.tile_pool(name="yp", bufs=2) as yp,
            tc.tile_pool(name="smallp", bufs=8) as smallp,
        ):
            # ---- two whole-plane loads carry x, weights and biases ----
            xts = xtp.tile([128, 2, N + 520], f16)
            xTr = xta.rearrange("(t p) n -> p t n", p=128)
            nc.scalar.dma_start(xts[:, 0, :], xTr[:, 0, :])
            nc.gpsimd.dma_start(xts[:, 1, :], xTr[:, 1, :])
            wq16 = lambda kt: xts[:, 0, N + 128 * kt : N + 128 * (kt + 1)]
            wk16 = lambda kt: xts[:, 0, N + 256 + 128 * kt : N + 256 + 128 * (kt + 1)]
            wv16 = lambda kt: xts[:, 1, N + 256 * kt : N + 256 * (kt + 1)]
            cb32 = xts[:, 0, N + 512 : N + 516].bitcast(f32)
            bq4s = cb32[:, 0:1]
            bk4s = cb32[:, 1:2]

            xr = xrp.tile([128, NQ // 128, CH], f32)
            nc.gpsimd.dma_start(
                xr[:], xres.rearrange("p (t c) -> p t c", c=CH)[:, :, :]
            )

            qt4 = qkp.tile([128, NQ], f16)
            kt4 = qkp.tile([128, N], f16)
            vaug = vp.tile([128, MT, CH2], bf16)
            # denominator-ones column + pad (written once; v copies fill 0:256)
            nc.gpsimd.memset(vaug[:, :, 256:257], 1.0)
            nc.gpsimd.memset(vaug[:, :, 257:258], 0.0)

            # ---- warmup (HAM clock ramp) + gamma broadcast + projections ----
            with (
                tc.tile_pool(name="psqk", bufs=3, space="PSUM") as psqk,
                tc.tile_pool(name="psv", bufs=4, space="PSUM") as psv,
            ):
                warm_sink = consts.tile([128, 1], f32)
                for w in range(2):
                    wt = psqk.tile([128, 512], f32, tag="pqk", name=f"warm{w}")
                    nc.tensor.matmul(
                        wt[:],
                        lhsT=wq16(0),
                        rhs=xts[:, 0, N : N + 512],
                        start=True,
                        stop=True,
                    )
                    if w == 1:
                        # keep the chain observable so it isn't dead-code
                        nc.vector.tensor_reduce(
                            warm_sink[:], wt[:], axis=mybir.AxisListType.X,
                            op=OP.max,
                        )
                # tiny exp so the ACT table set loads here (ACT is idle),
                # not right before the first real exp
                warm_exp = consts.tile([1, 2], f32)
                nc.scalar.activation(warm_exp[:], cb32[0:1, 0:2], AF.Exp)

                # ---- projections (fp16 in, f32 PSUM) ----
                # qT4[32a+d, n] = q[n, d] (own half), replicated over a
                for t in range(NS):
                    pq = psqk.tile([128, 512], f32, tag="pqk", name=f"pq{t}")
                    for kt in range(2):
                        nc.tensor.matmul(
                            pq[:],
                            lhsT=wq16(kt),
                            rhs=xts[:, kt, 512 * t : 512 * (t + 1)],
                            start=(kt == 0),
                            stop=(kt == 1),
                        )
                    nc.scalar.activation(
                        qt4[:, 512 * t : 512 * (t + 1)], pq[:],
                        AF.Identity, bias=bq4s,
                    )
                # kT4 over the full batch
                for t in range(N // 512):
                    pk = psqk.tile([128, 512], f32, tag="pqk", name=f"pk{t}")
                    for kt in range(2):
                        nc.tensor.matmul(
                            pk[:],
                            lhsT=wk16(kt),
                            rhs=xts[:, kt, 512 * t : 512 * (t + 1)],
                            start=(kt == 0),
                            stop=(kt == 1),
                        )
                    nc.scalar.activation(
                        kt4[:, 512 * t : 512 * (t + 1)], pk[:],
                        AF.Identity, bias=bk4s,
                    )
                # v natural layout [key, c]; bias folded into host residual
                for mt in range(MT):
                    pv = psv.tile([128, 512], f32, tag="pv", name=f"pv{mt}")
                    for kt in range(2):
                        nc.tensor.matmul(
                            pv[:, 0:CH],
                            lhsT=xts[:, kt, 128 * mt : 128 * (mt + 1)],
                            rhs=wv16(kt),
                            start=(kt == 0),
                            stop=(kt == 1),
                        )
                    nc.vector.tensor_copy(vaug[:, mt, 0:CH], pv[:, 0:CH])

            # ---- attention main loop ----
            # Per group of 4 key tiles: 4 concurrent row-tiled K=32 score
            # matmuls (one PSUM bank each, single-buffered), exp split
            # ACT/DVE, then 16 attn@v accumulation matmuls. PE emission
            # order is scores(g+1) before attnv(g) so the exp latency hides
            # under the previous group's attn@v stream.
            with (
                tc.tile_pool(name="pss", bufs=2, space="PSUM") as pss,
                tc.tile_pool(name="psa", bufs=1, space="PSUM") as psa,
            ):
                def scores_mm(ns, g, s):
                    # two concurrent K=32 matmuls on alternating strip pairs
                    # (even groups strips 0-1, odd groups 2-3 so LDWEIGHTS
                    # overlaps the previous group's matmuls)
                    for i in range(2):
                        st = 2 * (g % 2) + i
                        mt = 2 * g + i
                        nc.tensor.matmul(
                            s[:, i, :],
                            lhsT=kt4[32 * st : 32 * (st + 1), 128 * mt : 128 * (mt + 1)],
                            rhs=qt4[32 * st : 32 * (st + 1), 512 * ns : 512 * (ns + 1)],
                            start=True,
                            stop=True,
                            tile_position=(32 * st, 0),
                        )

                def attnv(e, g, acc):
                    # j-outer on the last group: each j's accumulation chain
                    # stops as early as possible so normalize(j) overlaps the
                    # remaining attn@v matmuls
                    order = (
                        [(j, i) for j in range(4) for i in range(2)]
                        if g == NG - 1
                        else [(j, i) for i in range(2) for j in range(4)]
                    )
                    for j, i in order:
                        mt = 2 * g + i
                        nc.tensor.matmul(
                            acc[:, j, 0:CH2],
                            lhsT=e[:, i, 128 * j : 128 * (j + 1)],
                            rhs=vaug[:, mt, :],
                            start=(g == 0 and i == 0),
                            stop=(g == NG - 1 and i == 1),
                        )

                def normalize(ns, acc):
                    # per-j evacuation: j's chain stopped early (j-outer
                    # final group), so copy+normalize overlap the remaining
                    # attn@v work and release acc banks sooner
                    accs = yp.tile([128, 4, CH2], f32, tag="accs")
                    yt = yp.tile([128, 4, CH], f32, tag="yt")
                    for j in range(4):
                        # acc evacuation on ACT (it has slack; DVE is the
                        # tighter engine in the mainloop)
                        nc.scalar.activation(
                            accs[:, j, :], acc[:, j, 0:CH2], AF.Identity
                        )
                        r = smallp.tile([128, 1], f32)
                        nc.vector.reciprocal(r[:], accs[:, j, CH : CH + 1])
                        nc.vector.scalar_tensor_tensor(
                            yt[:, j, :],
                            accs[:, j, 0:CH],
                            r[:, 0:1],
                            xr[:, 4 * ns + j, :],
                            op0=OP.mult,
                            op1=OP.add,
                        )
                    nc.gpsimd.dma_start(
                        y.rearrange("p (t c) -> p t c", c=CH)[
                            :, 4 * ns : 4 * (ns + 1), :
                        ],
                        yt[:],
                    )

                prev = None
                acc = None
                for ns in range(NS):
                    for g in range(NG):
                        s = pss.tile([128, 2, 512], f32, tag="s", name=f"s{ns}_{g}")
                        scores_mm(ns, g, s)
                        if prev is not None:
                            pe_, pns, pg_, pacc = prev
                            attnv(pe_, pg_, pacc)
                            if pg_ == NG - 1:
                                normalize(pns, pacc)
                        if g == 0:
                            acc = psa.tile([128, 4, 512], f32, tag="acc", name=f"acc{ns}")
                        e = expp.tile([128, 2, 512], bf16, tag="e", name=f"e{ns}_{g}")
                        # split across engines: ACT table-exp one tile, DVE
                        # Schraudolph the other
                        nc.scalar.activation(e[:, 0, :], s[:, 0, :], AF.Exp)
                        nc.vector.tensor_scalar(
                            e[:, 1, :].bitcast(i16), s[:, 1, :],
                            EXP_A, EXP_B, op0=OP.mult, op1=OP.add,
                        )
                        prev = (e, ns, g, acc)
                pe_, pns, pg_, pacc = prev
                attnv(pe_, pg_, pacc)
                normalize(pns, pacc)

    nc.compile()
    _COMPILED["nc"] = nc
    return nc


def _pack_consts(Wq, bq, Wk, bk, Wv, bv, gamma):
    """Pack constants into fp16 weight blob + fp32 small blob.

    cb16 (per partition p):
      [0:256)    Wq4 k-tiles: [wq4[p], wq4[p+128]]  (wq4 = tile(Wq, (1,4)))
      [256:512)  Wk4 k-tiles
      [512:1024) gamma*Wv k-tiles (256 each; gamma folded so the attn@v
                 accumulator is already scaled -- no gamma on device)
    cb32: [:,0]=bq4, [:,1]=bk4
    """
    g = np.float32(np.asarray(gamma).reshape(()))
    Wq4 = np.tile(np.asarray(Wq, np.float32), (1, 4)).astype(np.float16)
    Wk4 = np.tile(np.asarray(Wk, np.float32), (1, 4)).astype(np.float16)
    Wv16 = (g * np.asarray(Wv, np.float32)).astype(np.float16)

    cb16a = np.zeros((128, 512), np.float16)
    cb16b = np.zeros((128, 512), np.float16)
    for kt in range(2):
        cb16a[:, 128 * kt : 128 * (kt + 1)] = Wq4[128 * kt : 128 * (kt + 1), :]
        cb16a[:, 256 + 128 * kt : 256 + 128 * (kt + 1)] = Wk4[128 * kt : 128 * (kt + 1)]
        cb16b[:, 256 * kt : 256 * (kt + 1)] = Wv16[128 * kt : 128 * (kt + 1)]
    cb32 = np.zeros((128, 2), np.float32)
    cb32[:, 0] = np.tile(np.asarray(bq, np.float32), 4)
    cb32[:, 1] = np.tile(np.asarray(bk, np.float32), 4)
    return cb16a, cb16b, cb32


def _shard_inputs(x, Wq, bq, Wk, bk, Wv, bv, gamma):
    """Host-side prep: one input map per core."""
    xf = np.ascontiguousarray(x, dtype=np.float32).reshape(B, N, CH)
    x16 = xf.astype(np.float16)
    cb16a, cb16b, cb32 = _pack_consts(Wq, bq, Wk, bk, Wv, bv, gamma)
    g = np.float32(np.asarray(gamma).reshape(()))
    bv32 = np.asarray(bv, np.float32)

    in_maps = []
    for c in range(N_CORES):
        b, h = divmod(c, 2)
        own = slice(h * NQ, (h + 1) * NQ)
        other = slice((1 - h) * NQ, (2 - h) * NQ)
        xT = np.concatenate([x16[b, own].T, x16[b, other].T], axis=1)
        # augmented planes: x columns + weights + biases, 9KB rows
        xta = np.zeros((CH, N + 520), np.float16)
        xta[:, 0:N] = xT
        xta[0:128, N : N + 512] = cb16a
        xta[128:256, N : N + 512] = cb16b
        xta[0:128, N + 512 : N + 516] = cb32.view(np.float16)
        # partition-major residual with folded v bias
        xr = xf[b, own] + g * bv32[None, :]
        xr_p = xr.reshape(NQ // 128, 128, CH).transpose(1, 0, 2).reshape(128, -1)
        in_maps.append(
            {
                "xta": np.ascontiguousarray(xta),
                "xres": np.ascontiguousarray(xr_p),
            }
        )
    return in_maps


def _unshard(results, shape):
    out = np.empty((B, N, CH), np.float32)
    for c in range(N_CORES):
        b, h = divmod(c, 2)
        yp = (
            results[c]["y"]
            .reshape(128, NQ // 128, CH)
            .transpose(1, 0, 2)
            .reshape(NQ, CH)
        )
        out[b, h * NQ : (h + 1) * NQ, :] = yp
    return out.reshape(shape)


def kernel(x, Wq, bq, Wk, bk, Wv, bv, gamma):
    from concourse.bass_utils import run_bass_kernel_spmd

    nc = _build()
    in_maps = _shard_inputs(x, Wq, bq, Wk, bk, Wv, bv, gamma)
    res = run_bass_kernel_spmd(nc, in_maps, core_ids=list(range(N_CORES)))
    return _unshard(res.results, x.shape)
